# revision 7
# baseline (speedup 1.0000x reference)
"""LIF (leaky integrate-and-fire) forward scan on 8 Trainium2 NeuronCores.

Reference recurrence (per element, scan over T):
    m_t = v_{t-1} * tau + x_t
    y_t = (m_t - v_th > 0) ? 1.0 : 0.0
    v_t = m_t * (1 - y_t)          # hard reset on spike

x: [T=16, B=32, C=128, H=32, W=32] f32.  Data-parallel over B: each core
gets B_loc=4 batches; host pre-transposes each per-core block to
[T, C, F=4*H*W] so every per-step DMA is one fully-contiguous
16KiB-per-partition transfer.

Key optimization vs the 2-STT baseline: substitute v_t into the next
step's membrane update so v is never materialized:
    m_{t+1} = select(m_t <= v_th, tau*m_t, 0) + x_{t+1}
This is ONE custom-DVE instruction (LIF_M_STEP_ANT, registered below)
per step instead of two scalar_tensor_tensor passes -- bit-identical fp
semantics to the reference (tau*m is the same multiply, add order
unchanged, select arm exact).  Per step:
  DVE: m_{t+1} = LIF_M_STEP_ANT(x_{t+1}, m_t)        (skipped at t=0: m_0 = x_0)
  ACT: y_t = Sign(m_t - v_th) -> u8 (saturating convert maps -1 -> 0)
  DMA: x loads alternate the sync/scalar HWDGE rings; y stores ride the
       gpsimd SWDGE ring so the load rings stay pure.
"""

import sys

sys.path.insert(0, "/opt/trn_rl_repo")

from contextlib import ExitStack

import numpy as np

import concourse.bass as bass
import concourse.tile as tile
from concourse import bacc, mybir
from concourse.bass_utils import run_bass_kernel_spmd

# ---- custom DVE op: fused LIF membrane update ----------------------------
from concourse import dve_ops
from concourse.dve_ops import DveOp
from concourse.dve_spec import C0, C1, Spec, Src0, Src1, Zero, select

_LIF_OP_NAME = "LIF_M_STEP_ANT"


def _lif_ref(in0, in1, s0, s1, imm2):
    m = in1.astype(np.float32)
    return np.where(m <= s0, m * s1, 0.0).astype(np.float32) + in0.astype(np.float32)


LIF_M_STEP_ANT = DveOp(
    _LIF_OP_NAME,
    Spec(body=select(Src1 <= C0, Src1 * C1, Zero) + Src0, reference=_lif_ref),
    subdim=False,
    uops_sha={"v3": "2402d79924620f58", "v4": "d4561a7becc67430"},
)

if _LIF_OP_NAME not in dve_ops._SUB_OPCODE_FOR_NAME:
    dve_ops.OPS.append(LIF_M_STEP_ANT)
    dve_ops.CUSTOM_DVE_SPECS[_LIF_OP_NAME] = LIF_M_STEP_ANT.spec
    dve_ops._SUB_OPCODE_FOR_NAME[_LIF_OP_NAME] = (
        max(dve_ops._SUB_OPCODE_FOR_NAME.values()) + 1
    )
    assert dve_ops._SUB_OPCODE_FOR_NAME[_LIF_OP_NAME] < 0x20

# Hyperparameters (from the nn.Module)
V_TH = 1.0
TAU = 0.5

# Shapes (hardcoded per problem spec)
T, B, C, H, W = 16, 32, 128, 32, 32
N_CORES = 8
B_LOC = B // N_CORES           # 4 batches per core
S = H * W                      # 1024 spatial sites
F = B_LOC * S                  # 4096 free-dim sites per step

DT = mybir.dt.float32
U8 = mybir.dt.uint8


def build_kernel() -> bass.Bass:
    nc = bacc.Bacc(
        "TRN2", target_bir_lowering=False, debug=False, num_devices=N_CORES
    )
    x_d = nc.dram_tensor("x", [T, C, F], DT, kind="ExternalInput").ap()
    y_d = nc.dram_tensor("y", [T, C, F], U8, kind="ExternalOutput").ap()

    # Register a -V_TH const AP (activation bias needs a [128,1] SBUF const).
    _c = nc.alloc_sbuf_tensor(f"const-float32-{-V_TH}", [128, 1], DT)
    nc.gpsimd.memset(_c.ap(), -V_TH)
    nc.const_aps.aps[(DT, -V_TH)] = _c.ap()
    nc.all_engine_barrier()

    # Column split of the m-chain: DVE runs the fused custom op on
    # [0, D); the Pool (gpsimd) engine runs a stock tensor-op form on
    # [D, F) so the serial per-step DVE time drops below the DMA cadence.
    # (PC=0 disables the pool share: scalar_tensor_tensor is ISA-illegal
    # on Pool, and DVE's 5.5us/step hides under the ~6us DMA cadence.)
    PC = 0                     # pool column share
    D = F - PC

    with ExitStack() as ctx:
        tc = ctx.enter_context(tile.TileContext(nc))
        x_pool = ctx.enter_context(tc.tile_pool(name="x", bufs=7))
        m_pool = ctx.enter_context(tc.tile_pool(name="m", bufs=4))
        y_pool = ctx.enter_context(tc.tile_pool(name="y", bufs=3))
        v_pool = ctx.enter_context(tc.tile_pool(name="v", bufs=2))

        # Three DMA rings: sync + scalar HWDGE, gpsimd SWDGE.  Loads and
        # stores round-robin across all three so no single queue's
        # throughput (~360GB/s) caps the ~460GB/s steady-state demand.
        rings = (nc.sync, nc.scalar, nc.gpsimd)

        m_prev = None
        for t in range(T):
            xt = x_pool.tile([C, F], DT, tag="x", name=f"x{t}")
            if t < 2:
                # Ramp: split the first two loads across both HWDGE rings so
                # the m-chain starts as early as possible.
                half = F // 2
                nc.sync.dma_start(out=xt[:, 0:half], in_=x_d[t, :, 0:half])
                nc.scalar.dma_start(out=xt[:, half:F], in_=x_d[t, :, half:F])
            else:
                rings[t % 3].dma_start(out=xt[:], in_=x_d[t])

            if t == 0:
                mt = xt
            else:
                mt = m_pool.tile([C, F], DT, tag="m", name=f"m{t}")
                nc.vector._custom_dve(
                    LIF_M_STEP_ANT,
                    out=mt[:, 0:D],
                    in0=xt[:, 0:D],
                    in1=m_prev[:, 0:D],
                    s0=V_TH,
                    s1=TAU,
                )
                if PC:
                    vt = v_pool.tile([C, PC], DT, tag="v", name=f"v{t}")
                    nc.gpsimd.scalar_tensor_tensor(
                        vt[:], m_prev[:, D:F], V_TH, m_prev[:, D:F],
                        mybir.AluOpType.is_le, mybir.AluOpType.mult,
                    )
                    nc.gpsimd.scalar_tensor_tensor(
                        mt[:, D:F], vt[:], TAU, xt[:, D:F],
                        mybir.AluOpType.mult, mybir.AluOpType.add,
                    )
            m_prev = mt

            yt = y_pool.tile([C, F], U8, tag="y", name=f"y{t}")
            if t < T - 1:
                nc.scalar.activation(
                    yt[:], mt[:], mybir.ActivationFunctionType.Sign, bias=-V_TH
                )
                rings[(t + 2) % 3].dma_start(out=y_d[t], in_=yt[:])
            else:
                # Tail: split the final spike pass so the last store overlaps
                # the remaining activations.
                q = F // 4
                for j, (a, b) in enumerate(((0, q), (q, 2 * q), (2 * q, 3 * q), (3 * q, F))):
                    nc.scalar.activation(
                        yt[:, a:b], mt[:, a:b],
                        mybir.ActivationFunctionType.Sign, bias=-V_TH,
                    )
                    rings[j % 3].dma_start(out=y_d[t, :, a:b], in_=yt[:, a:b])
    nc.finalize()
    return nc


_NC_CACHE = None


def _get_nc():
    global _NC_CACHE
    if _NC_CACHE is None:
        _NC_CACHE = build_kernel()
    return _NC_CACHE


def _in_maps(x: np.ndarray) -> list[dict]:
    xf = np.asarray(x, dtype=np.float32).reshape(T, B, C, S)
    maps = []
    for k in range(N_CORES):
        blk = xf[:, k * B_LOC:(k + 1) * B_LOC]          # [T, B_loc, C, S]
        blk = np.ascontiguousarray(blk.transpose(0, 2, 1, 3))  # [T, C, B_loc, S]
        maps.append({"x": blk.reshape(T, C, F)})
    return maps


def kernel(x: np.ndarray) -> np.ndarray:
    assert x.shape == (T, B, C, H, W), x.shape
    in_dtype = x.dtype
    nc = _get_nc()
    in_maps = _in_maps(x)
    res = run_bass_kernel_spmd(nc, in_maps, list(range(N_CORES)))
    parts = []
    for k in range(N_CORES):
        yk = res.results[k]["y"].reshape(T, C, B_LOC, S).transpose(0, 2, 1, 3)
        parts.append(yk)                                # [T, B_loc, C, S]
    out = np.concatenate(parts, axis=1)                 # [T, B, C, S]
    return out.reshape(T, B, C, H, W).astype(in_dtype, copy=False)


if __name__ == "__main__":
    x = np.random.randn(T, B, C, H, W).astype(np.float32)
    y = kernel(x)
    print("out", y.shape, y.dtype, "spike rate", y.mean())


# revision 9
# speedup vs baseline: 1.2434x; 1.2434x over previous
"""LIF (leaky integrate-and-fire) forward scan on 8 Trainium2 NeuronCores.

Reference recurrence (per element, scan over T):
    m_t = v_{t-1} * tau + x_t
    y_t = (m_t - v_th > 0) ? 1.0 : 0.0
    v_t = m_t * (1 - y_t)          # hard reset on spike

x: [T=16, B=32, C=128, H=32, W=32] f32.  Data-parallel over B: each core
gets B_loc=4 batches; host pre-transposes each per-core block to
[T, C, F=4*H*W] so every per-step DMA is one fully-contiguous
16KiB-per-partition transfer.

Key optimization vs the 2-STT baseline: substitute v_t into the next
step's membrane update so v is never materialized:
    m_{t+1} = select(m_t <= v_th, tau*m_t, 0) + x_{t+1}
This is ONE custom-DVE instruction (LIF_M_STEP_ANT, registered below)
per step instead of two scalar_tensor_tensor passes -- bit-identical fp
semantics to the reference (tau*m is the same multiply, add order
unchanged, select arm exact).  Per step:
  DVE: m_{t+1} = LIF_M_STEP_ANT(x_{t+1}, m_t)        (skipped at t=0: m_0 = x_0)
  ACT: y_t = Sign(m_t - v_th) -> u8 (saturating convert maps -1 -> 0)
  DMA: x loads alternate the sync/scalar HWDGE rings; y stores ride the
       gpsimd SWDGE ring so the load rings stay pure.
"""

import sys

sys.path.insert(0, "/opt/trn_rl_repo")

from contextlib import ExitStack

import numpy as np

import concourse.bass as bass
import concourse.tile as tile
from concourse import bacc, mybir
from concourse.bass_utils import run_bass_kernel_spmd

# ---- custom DVE op: fused LIF membrane update ----------------------------
from concourse import dve_ops
from concourse.dve_ops import DveOp
from concourse.dve_spec import C0, C1, Spec, Src0, Src1, Zero, select

_LIF_OP_NAME = "LIF_M_STEP_ANT"


def _lif_ref(in0, in1, s0, s1, imm2):
    m = in1.astype(np.float32)
    return np.where(m <= s0, m * s1, 0.0).astype(np.float32) + in0.astype(np.float32)


LIF_M_STEP_ANT = DveOp(
    _LIF_OP_NAME,
    Spec(body=select(Src1 <= C0, Src1 * C1, Zero) + Src0, reference=_lif_ref),
    subdim=False,
    uops_sha={"v3": "2402d79924620f58", "v4": "d4561a7becc67430"},
)

if _LIF_OP_NAME not in dve_ops._SUB_OPCODE_FOR_NAME:
    dve_ops.OPS.append(LIF_M_STEP_ANT)
    dve_ops.CUSTOM_DVE_SPECS[_LIF_OP_NAME] = LIF_M_STEP_ANT.spec
    dve_ops._SUB_OPCODE_FOR_NAME[_LIF_OP_NAME] = (
        max(dve_ops._SUB_OPCODE_FOR_NAME.values()) + 1
    )
    assert dve_ops._SUB_OPCODE_FOR_NAME[_LIF_OP_NAME] < 0x20

# Hyperparameters (from the nn.Module)
V_TH = 1.0
TAU = 0.5

# Shapes (hardcoded per problem spec)
T, B, C, H, W = 16, 32, 128, 32, 32
N_CORES = 8
B_LOC = B // N_CORES           # 4 batches per core
S = H * W                      # 1024 spatial sites
F = B_LOC * S                  # 4096 free-dim sites per step

DT = mybir.dt.float32
U8 = mybir.dt.uint8


def build_kernel() -> bass.Bass:
    nc = bacc.Bacc(
        "TRN2", target_bir_lowering=False, debug=False, num_devices=N_CORES
    )
    x_d = nc.dram_tensor("x", [T, C, F], DT, kind="ExternalInput").ap()
    y_d = nc.dram_tensor("y", [T, C, F], U8, kind="ExternalOutput").ap()

    # Register a -V_TH const AP (activation bias needs a [128,1] SBUF const).
    _c = nc.alloc_sbuf_tensor(f"const-float32-{-V_TH}", [128, 1], DT)
    nc.gpsimd.memset(_c.ap(), -V_TH)
    nc.const_aps.aps[(DT, -V_TH)] = _c.ap()
    nc.all_engine_barrier()

    # Column split of the m-chain: DVE runs the fused custom op on
    # [0, D); the Pool (gpsimd) engine runs a stock tensor-op form on
    # [D, F) so the serial per-step DVE time drops below the DMA cadence.
    # (PC=0 disables the pool share: scalar_tensor_tensor is ISA-illegal
    # on Pool, and DVE's 5.5us/step hides under the ~6us DMA cadence.)
    PC = 0                     # pool column share
    D = F - PC

    with ExitStack() as ctx:
        tc = ctx.enter_context(tile.TileContext(nc))
        x_pool = ctx.enter_context(tc.tile_pool(name="x", bufs=8))
        m_pool = ctx.enter_context(tc.tile_pool(name="m", bufs=3))
        y_pool = ctx.enter_context(tc.tile_pool(name="y", bufs=4))
        v_pool = ctx.enter_context(tc.tile_pool(name="v", bufs=2))

        # Every x load is split half/half across the two HWDGE rings
        # (sync + scalar): each queue then only needs ~230GB/s of the
        # ~460GB/s steady-state demand, and arrival granularity is 1MiB.
        # Stores alternate the gpsimd SWDGE ring and the sync ring.
        half = F // 2

        m_prev = None
        for t in range(T):
            xt = x_pool.tile([C, F], DT, tag="x", name=f"x{t}")
            nc.sync.dma_start(out=xt[:, 0:half], in_=x_d[t, :, 0:half])
            nc.scalar.dma_start(out=xt[:, half:F], in_=x_d[t, :, half:F])

            if t == 0:
                mt = xt
            else:
                mt = m_pool.tile([C, F], DT, tag="m", name=f"m{t}")
                nc.vector._custom_dve(
                    LIF_M_STEP_ANT,
                    out=mt[:, 0:D],
                    in0=xt[:, 0:D],
                    in1=m_prev[:, 0:D],
                    s0=V_TH,
                    s1=TAU,
                )
                if PC:
                    vt = v_pool.tile([C, PC], DT, tag="v", name=f"v{t}")
                    nc.gpsimd.scalar_tensor_tensor(
                        vt[:], m_prev[:, D:F], V_TH, m_prev[:, D:F],
                        mybir.AluOpType.is_le, mybir.AluOpType.mult,
                    )
                    nc.gpsimd.scalar_tensor_tensor(
                        mt[:, D:F], vt[:], TAU, xt[:, D:F],
                        mybir.AluOpType.mult, mybir.AluOpType.add,
                    )
            m_prev = mt

            yt = y_pool.tile([C, F], U8, tag="y", name=f"y{t}")
            if t < T - 1:
                nc.scalar.activation(
                    yt[:], mt[:], mybir.ActivationFunctionType.Sign, bias=-V_TH
                )
                ring = nc.gpsimd if (t % 2 == 0) else nc.sync
                ring.dma_start(out=y_d[t], in_=yt[:])
            else:
                # Tail: split the final spike pass so the last store overlaps
                # the remaining activations.
                q = F // 4
                srings = (nc.gpsimd, nc.sync, nc.gpsimd, nc.sync)
                for j, (a, b) in enumerate(((0, q), (q, 2 * q), (2 * q, 3 * q), (3 * q, F))):
                    nc.scalar.activation(
                        yt[:, a:b], mt[:, a:b],
                        mybir.ActivationFunctionType.Sign, bias=-V_TH,
                    )
                    srings[j].dma_start(out=y_d[t, :, a:b], in_=yt[:, a:b])
    nc.finalize()
    return nc


_NC_CACHE = None


def _get_nc():
    global _NC_CACHE
    if _NC_CACHE is None:
        _NC_CACHE = build_kernel()
    return _NC_CACHE


def _in_maps(x: np.ndarray) -> list[dict]:
    xf = np.asarray(x, dtype=np.float32).reshape(T, B, C, S)
    maps = []
    for k in range(N_CORES):
        blk = xf[:, k * B_LOC:(k + 1) * B_LOC]          # [T, B_loc, C, S]
        blk = np.ascontiguousarray(blk.transpose(0, 2, 1, 3))  # [T, C, B_loc, S]
        maps.append({"x": blk.reshape(T, C, F)})
    return maps


def kernel(x: np.ndarray) -> np.ndarray:
    assert x.shape == (T, B, C, H, W), x.shape
    in_dtype = x.dtype
    nc = _get_nc()
    in_maps = _in_maps(x)
    res = run_bass_kernel_spmd(nc, in_maps, list(range(N_CORES)))
    parts = []
    for k in range(N_CORES):
        yk = res.results[k]["y"].reshape(T, C, B_LOC, S).transpose(0, 2, 1, 3)
        parts.append(yk)                                # [T, B_loc, C, S]
    out = np.concatenate(parts, axis=1)                 # [T, B, C, S]
    return out.reshape(T, B, C, H, W).astype(in_dtype, copy=False)


if __name__ == "__main__":
    x = np.random.randn(T, B, C, H, W).astype(np.float32)
    y = kernel(x)
    print("out", y.shape, y.dtype, "spike rate", y.mean())
